# revision 23
# baseline (speedup 1.0000x reference)
"""TRN2 Bass kernel for ConvNeXt-MLP + parallel top-2-of-3 LoRA-MoE.

Data-parallel over the token dim across 8 NeuronCores (12544 tokens ->
1568/core). All weights replicated. Per core, everything is computed in
feature-major ("transposed") layout: activations live in SBUF as
[features_on_partitions, tokens_on_free_dim]; the host transposes x in and
the output back out.

All matmuls run in bf16 (1 cycle/row on the PE; the 2-byte LDWEIGHTS hides
under the matmul, unlike the 4-byte f32r weight load). w1 and w2 are fully
SBUF-resident in bf16 (9.4 MB), so the hidden dim is a single 24-chunk pass
per token tile with the output accumulating entirely in PSUM.

Router exactness: bf16 alone flips the top-2 selection on ~8 near-tie
tokens (each flip is a ~0.1 abs output error). The kernel therefore
computes logits as x_hi@rw_hi + x_hi@rw_lo + x_lo@rw_hi with bf16 hi/lo
splits of both operands (max logit err ~2e-5 vs the f32 reference, smallest
top-2 margin in-distribution is 5.3e-5 -> selection is bit-identical).
rw_lo/lora-down live at PSUM quadrant offsets 32/64 (engine partition-base
rule). Softmax + top-2 + renormalize run fully batched on DVE via one
32x32 stream transpose each way.

Scheduling: one need-ordered DMA stream on the sync queue (xt0, rwd, xlo0,
then w1/w2 j-chunks interleaved with the remaining xt/xlo tiles in exact
consumption order - the queues fair-share HBM bandwidth, so priority =
order). Phase A for tiles 1-3 is interleaved INTO tile 0's j-loop (at
j=4/8/12) so their x DMAs arrive under compute. PSUM->SBUF output copies
run on the otherwise-idle Vector engine, interleaved with the LoRA-up
matmuls per output chunk.
"""

import numpy as np
import ml_dtypes

import concourse.bacc as bacc
import concourse.mybir as mybir
import concourse.tile as tile
from concourse.bass_utils import run_bass_kernel_spmd

F32 = mybir.dt.float32
BF16 = mybir.dt.bfloat16
AF = mybir.ActivationFunctionType
ALU = mybir.AluOpType
AX = mybir.AxisListType

NCORES = 8
B, N, D = 64, 196, 768
T = B * N                  # 12544 tokens total
TC = T // NCORES           # 1568 tokens per core
HID = 4 * D                # 3072
E, R = 3, 8
ER = E * R                 # 24
DC = D // 128              # 6 input-feature chunks
HC = HID // 128            # 24 hidden chunks
MC = D // 128              # 6 output chunks
NT_SIZES = [448, 448, 448, 224]    # token tiles per core (sum = 1568)
NT_OFF = [0, 448, 896, 1344]
NBLK = TC // 32            # 49 32-token blocks for the stream transpose
# w1/w2 are DMA'd in j-chunks in consumption order, interleaved with the
# xt/xlo tiles of later token tiles
WCHUNKS = [(0, 2), (2, 4), (4, 8), (8, 12), (12, 17), (17, 24)]

_cache = {}


def _build():
    nc = bacc.Bacc("TRN2", target_bir_lowering=False, debug=False)

    # all inputs are host-packed to the exact SBUF layout so every DMA is
    # a straight [128, X] copy with multi-KB rows at full HBM bandwidth
    xt_d = nc.dram_tensor("xt", [128, DC * TC], BF16, kind="ExternalInput")
    xlo_d = nc.dram_tensor("xlo", [128, DC * TC], BF16, kind="ExternalInput")
    w1_d = nc.dram_tensor("w1", [128, HC * DC * 128], BF16,
                          kind="ExternalInput")
    w2_d = nc.dram_tensor("w2", [128, HC * D], BF16, kind="ExternalInput")
    wu_d = nc.dram_tensor("wu", [ER, D], BF16, kind="ExternalInput")
    b1_d = nc.dram_tensor("b1r", [128, HC], F32, kind="ExternalInput")
    b2_d = nc.dram_tensor("b2r", [128, MC], F32, kind="ExternalInput")
    rwd_d = nc.dram_tensor("rwd", [128, DC * 96], BF16, kind="ExternalInput")
    rb_d = nc.dram_tensor("rb", [E, 1], F32, kind="ExternalInput")
    bx_d = nc.dram_tensor("bexp", [E, ER], BF16, kind="ExternalInput")
    out_d = nc.dram_tensor("outT", [128, MC * TC], F32,
                           kind="ExternalOutput")

    with tile.TileContext(nc) as tc:
        with (
            tc.tile_pool(name="const", bufs=1) as cp,
            tc.tile_pool(name="big", bufs=1) as bp,
            tc.tile_pool(name="hbuf", bufs=3) as hp,
            tc.tile_pool(name="osb", bufs=2) as op,
            tc.tile_pool(name="psO", bufs=1, space="PSUM") as psO,
            tc.tile_pool(name="psH", bufs=2, space="PSUM") as psH,
        ):
            # ---- tiles ----
            xts = [bp.tile([128, DC * n], BF16, tag=f"xt{i}", name=f"xt{i}")
                   for i, n in enumerate(NT_SIZES)]
            xlos = [bp.tile([128, DC * n], BF16, tag=f"xlo{i}",
                            name=f"xlo{i}")
                    for i, n in enumerate(NT_SIZES)]
            rwd = cp.tile([128, DC * 96], BF16, tag="rwd")
            wu = cp.tile([ER, D], BF16, tag="wu")
            b1 = cp.tile([128, HC], F32, tag="b1")
            b2 = cp.tile([128, MC], F32, tag="b2")
            rb = cp.tile([E, 1], F32, tag="rb")
            bx = cp.tile([E, ER], BF16, tag="bx")
            w1s = bp.tile([128, HC * DC * 128], BF16, tag="w1s")
            w2s = bp.tile([128, HC * D], BF16, tag="w2s")
            w2v = w2s[:].rearrange("p (j f) -> p j f", j=HC)
            lgT = bp.tile([32, TC], F32, tag="lgT")
            acts = bp.tile([ER, TC], F32, tag="acts")
            scaled = bp.tile([ER, TC], BF16, tag="scaled")

            def load_x(i):
                lo = DC * NT_OFF[i]
                hi = lo + DC * NT_SIZES[i]
                nc.sync.dma_start(xts[i][:], xt_d[:, lo:hi])
                nc.sync.dma_start(xlos[i][:], xlo_d[:, lo:hi])

            def load_w(q):
                j0, j1 = WCHUNKS[q]
                nc.sync.dma_start(w1s[:, j0 * DC * 128:j1 * DC * 128],
                                  w1_d[:, j0 * DC * 128:j1 * DC * 128])
                nc.sync.dma_start(w2s[:, j0 * D:j1 * D],
                                  w2_d[:, j0 * D:j1 * D])

            # need-ordered single DMA stream (sync); tiny consts on gpsimd
            nc.sync.dma_start(xts[0][:], xt_d[:, 0:DC * NT_SIZES[0]])
            nc.sync.dma_start(rwd[:], rwd_d[:])
            nc.sync.dma_start(xlos[0][:], xlo_d[:, 0:DC * NT_SIZES[0]])
            nc.sync.dma_start(w1s[:, 0:2 * DC * 128],
                              w1_d[:, 0:2 * DC * 128])
            nc.sync.dma_start(w2s[:, 0:2 * D], w2_d[:, 0:2 * D])
            nc.gpsimd.dma_start(rb[:], rb_d[:])
            nc.gpsimd.dma_start(b1[:], b1_d[:])
            nc.gpsimd.dma_start(b2[:], b2_d[:])
            nc.gpsimd.dma_start(bx[:], bx_d[:])
            nc.gpsimd.dma_start(wu[:], wu_d[:])
            load_w(1)
            load_x(1)
            load_w(2)
            load_x(2)
            load_w(3)
            load_x(3)
            load_w(4)
            load_w(5)

            # warm up the PE DVFS ramp with dependency-free tiny matmuls so
            # the first real matmuls run at full clock
            warm = cp.tile([16, 16], BF16, tag="warm")
            nc.vector.memset(warm[:], 0.0)
            wps = psH.tile([16, 512], F32, tag="h", name="warm_ps")
            for k in range(10):
                nc.tensor.matmul(wps[:16, k * 16:k * 16 + 16], warm[:],
                                 warm[:], start=True, stop=True,
                                 skip_group_check=True)

            nc.vector.memset(lgT[:], 0.0)

            # ---- phase A (per tile): router logits + LoRA activations ----
            def phase_a(i):
                n, t0 = NT_SIZES[i], NT_OFF[i]
                dn27 = psH.tile([88, 512], F32, tag="h", name=f"dn27_{i}")
                for c in range(DC):
                    nc.tensor.matmul(
                        dn27[:, :n],
                        rwd[:, c * 96:c * 96 + 88],
                        xts[i][:, c * n:(c + 1) * n],
                        start=(c == 0), stop=(c == DC - 1),
                    )
                for c in range(DC):
                    nc.tensor.matmul(
                        dn27[:3, :n],
                        rwd[:, c * 96:c * 96 + 3],
                        xlos[i][:, c * n:(c + 1) * n],
                        start=False, stop=(c == DC - 1),
                        skip_group_check=True,
                    )
                nc.vector.tensor_scalar_add(lgT[:E, t0:t0 + n],
                                            dn27[:E, :n], rb[:])
                nc.vector.tensor_add(lgT[:E, t0:t0 + n],
                                     lgT[:E, t0:t0 + n],
                                     dn27[32:32 + E, :n])
                nc.scalar.activation(acts[:, t0:t0 + n], dn27[64:, :n],
                                     AF.Gelu)

            # batched softmax + top-2-of-3 renormalized combine weights:
            # comb_e = (p_e > p_min) * p_e / ((sum - min)/sum + 1e-6) / sum
            def softmax_block():
                lgtok = bp.tile([32, TC], F32, tag="lgtok")
                nc.vector.transpose(lgtok[:], lgT[:])
                ltv = lgtok[:].rearrange("p (b q) -> p b q", b=NBLK)[:, :, :E]
                probs = bp.tile([32, NBLK * E], F32, tag="probs")
                prv = probs[:].rearrange("p (b q) -> p b q", b=NBLK)
                nc.scalar.activation(prv, ltv, AF.Exp)
                ssum = bp.tile([32, NBLK], F32, tag="ssum")
                nc.vector.tensor_reduce(ssum[:], prv, axis=AX.X, op=ALU.add)
                pmin = bp.tile([32, NBLK], F32, tag="pmin")
                nc.vector.tensor_reduce(pmin[:], prv, axis=AX.X, op=ALU.min)
                rs = bp.tile([32, NBLK], F32, tag="rs")
                nc.vector.reciprocal(rs[:], ssum[:])
                den = bp.tile([32, NBLK], F32, tag="den")
                nc.vector.tensor_sub(den[:], ssum[:], pmin[:])
                nc.vector.tensor_mul(den[:], den[:], rs[:])
                nc.vector.tensor_scalar_add(den[:], den[:], 1e-6)
                invd = bp.tile([32, NBLK], F32, tag="invd")
                nc.vector.reciprocal(invd[:], den[:])
                t1 = bp.tile([32, NBLK], F32, tag="t1")
                nc.vector.tensor_mul(t1[:], rs[:], invd[:])
                combt = bp.tile([32, NBLK * 32], BF16, tag="combt")
                cbv = combt[:].rearrange("p (b q) -> p b q", b=NBLK)[:, :, :E]
                mask = bp.tile([32, NBLK * E], F32, tag="mask")
                mkv = mask[:].rearrange("p (b q) -> p b q", b=NBLK)
                pminb = pmin[:].unsqueeze(2).broadcast_to([32, NBLK, E])
                nc.vector.tensor_tensor(mkv, prv, pminb, op=ALU.is_gt)
                nc.vector.tensor_mul(mkv, mkv, prv)
                t1b = t1[:].unsqueeze(2).broadcast_to([32, NBLK, E])
                nc.vector.tensor_tensor(cbv, mkv, t1b, op=ALU.mult)
                combT = bp.tile([32, TC], BF16, tag="combT")
                nc.vector.transpose(combT[:], combt[:])
                return combT

            def expand_block(combT):
                for i2, n2 in enumerate(NT_SIZES):
                    tq = NT_OFF[i2]
                    ex = psH.tile([ER, 512], F32, tag="h", name=f"ex_{i2}")
                    nc.tensor.matmul(ex[:, :n2], bx[:],
                                     combT[:E, tq:tq + n2],
                                     start=True, stop=True)
                    nc.vector.tensor_mul(scaled[:, tq:tq + n2],
                                         acts[:, tq:tq + n2], ex[:, :n2])

            # ---- phase B: base MLP, out accumulates across all 24 j in
            # PSUM; phase A of tiles 1-3 and the softmax/expand interleave
            # into tile 0's j-loop so their inputs arrive under compute ----
            combT = None
            phase_a(0)
            for nt, n in enumerate(NT_SIZES):
                t0 = NT_OFF[nt]
                outp = [psO.tile([128, 512], F32, tag=f"out{m}",
                                 name=f"out{m}_{nt}")
                        for m in range(MC)]
                hsb_prev = None
                for j in range(HC + 1):
                    if nt == 0 and j in (4, 8, 12):
                        phase_a(j // 4)
                        if j == 12:
                            combT = softmax_block()
                    if nt == 0 and j == 19:
                        expand_block(combT)
                    if j < HC:
                        hps = psH.tile([128, 512], F32, tag="h",
                                       name=f"h_{nt}_{j}")
                        for c in range(DC):
                            o = (j * DC + c) * 128
                            nc.tensor.matmul(
                                hps[:, :n],
                                w1s[:, o:o + 128],
                                xts[nt][:, c * n:(c + 1) * n],
                                start=(c == 0), stop=(c == DC - 1),
                            )
                        hsb = hp.tile([128, 512], BF16, tag="hs",
                                      name=f"hs_{nt}_{j}")
                        nc.scalar.activation(
                            hsb[:, :n], hps[:, :n], AF.Gelu,
                            bias=b1[:, j:j + 1],
                        )
                    if j >= 1:
                        jj = j - 1
                        for m in range(MC):
                            nc.tensor.matmul(
                                outp[m][:, :n],
                                w2v[:, jj, m * 128:(m + 1) * 128],
                                hsb_prev[:, :n],
                                start=(jj == 0), stop=False,
                            )
                    hsb_prev = hsb
                # LoRA-up closes each PSUM accumulation group; the PSUM
                # evacuation + bias runs on DVE right behind each chunk
                last = nt == len(NT_SIZES) - 1
                osb = op.tile([128, MC * 512], F32, tag="osb",
                              name=f"osb_{nt}")
                for m in range(MC):
                    nc.tensor.matmul(
                        outp[m][:, :n],
                        wu[:, m * 128:(m + 1) * 128],
                        scaled[:, t0:t0 + n],
                        start=False, stop=True,
                    )
                    if last and m % 2 == 1:
                        nc.scalar.activation(
                            osb[:, m * 512:m * 512 + n], outp[m][:, :n],
                            AF.Identity, bias=b2[:, m:m + 1],
                        )
                    else:
                        nc.vector.tensor_scalar_add(
                            osb[:, m * 512:m * 512 + n], outp[m][:, :n],
                            b2[:, m:m + 1],
                        )
                oo = MC * t0
                odv = out_d[:, oo:oo + MC * n].rearrange(
                    "p (m t) -> p m t", m=MC)
                osv = osb[:].rearrange("p (m t) -> p m t", m=MC)
                if last:
                    nc.sync.dma_start(odv[:, :MC // 2, :],
                                      osv[:, :MC // 2, :n])
                    nc.sync.dma_start(odv[:, MC // 2:, :],
                                      osv[:, MC // 2:, :n])
                else:
                    nc.sync.dma_start(odv[:, :, :], osv[:, :, :n])

    nc.compile()
    return nc


def _pack_rwd(router_w, w_down):
    rw = np.asarray(router_w, np.float32)
    rw_hi = rw.astype(ml_dtypes.bfloat16).astype(np.float32)
    rwd = np.zeros((D, 96), ml_dtypes.bfloat16)
    rwd[:, :E] = rw_hi
    rwd[:, 32:32 + E] = rw - rw_hi
    rwd[:, 64:88] = np.asarray(w_down, np.float32).transpose(1, 0, 2).reshape(D, ER)
    # pack to SBUF layout [p, c, e]
    return np.ascontiguousarray(
        rwd.reshape(DC, 128, 96).transpose(1, 0, 2).reshape(128, DC * 96))


def _bf16(a):
    return np.ascontiguousarray(
        np.asarray(a, np.float32).astype(ml_dtypes.bfloat16))


def _pack_x(xT):
    # [D, TC] -> tile-major [128, sum(DC*n)]: per tile [p, c, t] contiguous
    blocks = []
    for i, n in enumerate(NT_SIZES):
        t0 = NT_OFF[i]
        blk = xT[:, t0:t0 + n].reshape(DC, 128, n).transpose(1, 0, 2)
        blocks.append(blk.reshape(128, DC * n))
    return np.ascontiguousarray(np.concatenate(blocks, axis=1))


def _prep_inputs(x, w1, b1, w2, b2, router_w, router_b, w_down, w_up):
    x = np.asarray(x, dtype=np.float32)
    xT = x.reshape(T, D).T  # [D, T]
    w1p = _bf16(w1).reshape(DC, 128, HC, 128).transpose(1, 2, 0, 3)
    w2p = _bf16(w2).reshape(HC, 128, D).transpose(1, 0, 2)
    common = {
        "w1": np.ascontiguousarray(w1p.reshape(128, HC * DC * 128)),
        "w2": np.ascontiguousarray(w2p.reshape(128, HC * D)),
        "wu": _bf16(np.asarray(w_up, np.float32).reshape(ER, D)),
        "b1r": np.ascontiguousarray(
            np.asarray(b1, np.float32).reshape(HC, 128).T),
        "b2r": np.ascontiguousarray(
            np.asarray(b2, np.float32).reshape(MC, 128).T),
        "rwd": _pack_rwd(router_w, w_down),
        "rb": np.ascontiguousarray(
            np.asarray(router_b, np.float32).reshape(E, 1)),
        "bexp": _bf16(np.repeat(np.eye(E, dtype=np.float32), R, axis=1)),
    }
    xT_hi = xT.astype(ml_dtypes.bfloat16)
    xT_lo = (xT - xT_hi.astype(np.float32)).astype(ml_dtypes.bfloat16)
    in_maps = []
    for c in range(NCORES):
        m = dict(common)
        m["xt"] = _pack_x(xT_hi[:, c * TC:(c + 1) * TC])
        m["xlo"] = _pack_x(xT_lo[:, c * TC:(c + 1) * TC])
        in_maps.append(m)
    return in_maps


def _run(inputs, trace=False):
    if "nc" not in _cache:
        _cache["nc"] = _build()
    nc = _cache["nc"]
    in_maps = _prep_inputs(**inputs)
    res = run_bass_kernel_spmd(nc, in_maps, core_ids=list(range(NCORES)),
                               trace=trace)
    # unpack tile-major [128, MC*TC] per core -> [D, T] -> [B, N, D]
    cols = []
    for c in range(NCORES):
        arr = res.results[c]["outT"]
        for i, n in enumerate(NT_SIZES):
            oo = MC * NT_OFF[i]
            blk = arr[:, oo:oo + MC * n].reshape(128, MC, n)
            cols.append(blk.transpose(1, 0, 2).reshape(D, n))
    outT = np.concatenate(cols, axis=1)  # [D, T]
    out = np.ascontiguousarray(outT.T).reshape(B, N, D).astype(np.float32)
    return out, res


def kernel(**inputs):
    return _run(inputs)[0]


# revision 24
# speedup vs baseline: 1.0033x; 1.0033x over previous
"""TRN2 Bass kernel for ConvNeXt-MLP + parallel top-2-of-3 LoRA-MoE.

Data-parallel over the token dim across 8 NeuronCores (12544 tokens ->
1568/core). All weights replicated. Per core, everything is computed in
feature-major ("transposed") layout: activations live in SBUF as
[features_on_partitions, tokens_on_free_dim]; the host transposes x in and
the output back out.

All matmuls run in bf16 (1 cycle/row on the PE; the 2-byte LDWEIGHTS hides
under the matmul, unlike the 4-byte f32r weight load). w1 and w2 are fully
SBUF-resident in bf16 (9.4 MB), so the hidden dim is a single 24-chunk pass
per token tile with the output accumulating entirely in PSUM.

Router exactness: bf16 alone flips the top-2 selection on ~8 near-tie
tokens (each flip is a ~0.1 abs output error). The kernel therefore
computes logits as x_hi@rw_hi + x_hi@rw_lo + x_lo@rw_hi with bf16 hi/lo
splits of both operands (max logit err ~2e-5 vs the f32 reference, smallest
top-2 margin in-distribution is 5.3e-5 -> selection is bit-identical).
rw_lo/lora-down live at PSUM quadrant offsets 32/64 (engine partition-base
rule). Softmax + top-2 + renormalize run fully batched on DVE via one
32x32 stream transpose each way.

Scheduling: one need-ordered DMA stream on the sync queue (xt0, rwd, xlo0,
then w1/w2 j-chunks interleaved with the remaining xt/xlo tiles in exact
consumption order - the queues fair-share HBM bandwidth, so priority =
order). Phase A for tiles 1-3 is interleaved INTO tile 0's j-loop (at
j=4/8/12) so their x DMAs arrive under compute. PSUM->SBUF output copies
run on the otherwise-idle Vector engine, interleaved with the LoRA-up
matmuls per output chunk.
"""

import numpy as np
import ml_dtypes

import concourse.bacc as bacc
import concourse.mybir as mybir
import concourse.tile as tile
from concourse.bass_utils import run_bass_kernel_spmd

F32 = mybir.dt.float32
BF16 = mybir.dt.bfloat16
AF = mybir.ActivationFunctionType
ALU = mybir.AluOpType
AX = mybir.AxisListType

NCORES = 8
B, N, D = 64, 196, 768
T = B * N                  # 12544 tokens total
TC = T // NCORES           # 1568 tokens per core
HID = 4 * D                # 3072
E, R = 3, 8
ER = E * R                 # 24
DC = D // 128              # 6 input-feature chunks
HC = HID // 128            # 24 hidden chunks
MC = D // 128              # 6 output chunks
NT_SIZES = [448, 448, 448, 224]    # token tiles per core (sum = 1568)
NT_OFF = [0, 448, 896, 1344]
NBLK = TC // 32            # 49 32-token blocks for the stream transpose
# w1/w2 are DMA'd in j-chunks in consumption order, interleaved with the
# xt/xlo tiles of later token tiles
WCHUNKS = [(0, 2), (2, 4), (4, 8), (8, 12), (12, 17), (17, 24)]

_cache = {}


def _build():
    nc = bacc.Bacc("TRN2", target_bir_lowering=False, debug=False)

    # all inputs are host-packed to the exact SBUF layout so every DMA is
    # a straight [128, X] copy with multi-KB rows at full HBM bandwidth
    xt_d = nc.dram_tensor("xt", [128, DC * TC], BF16, kind="ExternalInput")
    xlo_d = nc.dram_tensor("xlo", [128, DC * TC], BF16, kind="ExternalInput")
    w1_d = nc.dram_tensor("w1", [128, HC * DC * 128], BF16,
                          kind="ExternalInput")
    w2_d = nc.dram_tensor("w2", [128, HC * D], BF16, kind="ExternalInput")
    wu_d = nc.dram_tensor("wu", [ER, D], BF16, kind="ExternalInput")
    b1_d = nc.dram_tensor("b1r", [128, HC], F32, kind="ExternalInput")
    b2_d = nc.dram_tensor("b2r", [128, MC], F32, kind="ExternalInput")
    rwd_d = nc.dram_tensor("rwd", [128, DC * 96], BF16, kind="ExternalInput")
    rb_d = nc.dram_tensor("rb", [E, 1], F32, kind="ExternalInput")
    bx_d = nc.dram_tensor("bexp", [E, ER], BF16, kind="ExternalInput")
    out_d = nc.dram_tensor("outT", [128, MC * TC], F32,
                           kind="ExternalOutput")

    with tile.TileContext(nc) as tc:
        with (
            tc.tile_pool(name="const", bufs=1) as cp,
            tc.tile_pool(name="big", bufs=1) as bp,
            tc.tile_pool(name="hbuf", bufs=3) as hp,
            tc.tile_pool(name="osb", bufs=2) as op,
            tc.tile_pool(name="psO", bufs=1, space="PSUM") as psO,
            tc.tile_pool(name="psH", bufs=2, space="PSUM") as psH,
        ):
            # ---- tiles ----
            xts = [bp.tile([128, DC * n], BF16, tag=f"xt{i}", name=f"xt{i}")
                   for i, n in enumerate(NT_SIZES)]
            xlos = [bp.tile([128, DC * n], BF16, tag=f"xlo{i}",
                            name=f"xlo{i}")
                    for i, n in enumerate(NT_SIZES)]
            rwd = cp.tile([128, DC * 96], BF16, tag="rwd")
            wu = cp.tile([ER, D], BF16, tag="wu")
            b1 = cp.tile([128, HC], F32, tag="b1")
            b2 = cp.tile([128, MC], F32, tag="b2")
            rb = cp.tile([E, 1], F32, tag="rb")
            bx = cp.tile([E, ER], BF16, tag="bx")
            w1s = bp.tile([128, HC * DC * 128], BF16, tag="w1s")
            w2s = bp.tile([128, HC * D], BF16, tag="w2s")
            w2v = w2s[:].rearrange("p (j f) -> p j f", j=HC)
            lgT = bp.tile([32, TC], F32, tag="lgT")
            acts = bp.tile([ER, TC], F32, tag="acts")
            scaled = bp.tile([ER, TC], BF16, tag="scaled")

            def load_x(i):
                lo = DC * NT_OFF[i]
                hi = lo + DC * NT_SIZES[i]
                nc.sync.dma_start(xts[i][:], xt_d[:, lo:hi])
                nc.sync.dma_start(xlos[i][:], xlo_d[:, lo:hi])

            def load_w(q):
                j0, j1 = WCHUNKS[q]
                nc.sync.dma_start(w1s[:, j0 * DC * 128:j1 * DC * 128],
                                  w1_d[:, j0 * DC * 128:j1 * DC * 128])
                nc.sync.dma_start(w2s[:, j0 * D:j1 * D],
                                  w2_d[:, j0 * D:j1 * D])

            # need-ordered single DMA stream (sync); tiny consts on gpsimd.
            # xt0/xlo0 stream in c-chunk pieces so phase A's contraction loop
            # starts on the first chunk right as the PE warmup ends
            n0 = NT_SIZES[0]
            nc.sync.dma_start(rwd[:], rwd_d[:])
            for c0, c1 in ((0, 2), (2, 4), (4, 6)):
                nc.sync.dma_start(xts[0][:, c0 * n0:c1 * n0],
                                  xt_d[:, c0 * n0:c1 * n0])
            for c0, c1 in ((0, 3), (3, 6)):
                nc.sync.dma_start(xlos[0][:, c0 * n0:c1 * n0],
                                  xlo_d[:, c0 * n0:c1 * n0])
            nc.sync.dma_start(w1s[:, 0:2 * DC * 128],
                              w1_d[:, 0:2 * DC * 128])
            nc.sync.dma_start(w2s[:, 0:2 * D], w2_d[:, 0:2 * D])
            nc.gpsimd.dma_start(rb[:], rb_d[:])
            nc.gpsimd.dma_start(b1[:], b1_d[:])
            nc.gpsimd.dma_start(b2[:], b2_d[:])
            nc.gpsimd.dma_start(bx[:], bx_d[:])
            nc.gpsimd.dma_start(wu[:], wu_d[:])
            load_w(1)
            load_x(1)
            load_w(2)
            load_x(2)
            load_w(3)
            load_x(3)
            load_w(4)
            load_w(5)

            # warm up the PE DVFS ramp with dependency-free tiny matmuls so
            # the first real matmuls run at full clock
            warm = cp.tile([16, 16], BF16, tag="warm")
            nc.vector.memset(warm[:], 0.0)
            wps = psH.tile([16, 512], F32, tag="h", name="warm_ps")
            for k in range(10):
                nc.tensor.matmul(wps[:16, k * 16:k * 16 + 16], warm[:],
                                 warm[:], start=True, stop=True,
                                 skip_group_check=True)

            nc.vector.memset(lgT[:], 0.0)

            # ---- phase A (per tile): router logits + LoRA activations ----
            def phase_a(i):
                n, t0 = NT_SIZES[i], NT_OFF[i]
                dn27 = psH.tile([88, 512], F32, tag="h", name=f"dn27_{i}")
                for c in range(DC):
                    nc.tensor.matmul(
                        dn27[:, :n],
                        rwd[:, c * 96:c * 96 + 88],
                        xts[i][:, c * n:(c + 1) * n],
                        start=(c == 0), stop=(c == DC - 1),
                    )
                for c in range(DC):
                    nc.tensor.matmul(
                        dn27[:3, :n],
                        rwd[:, c * 96:c * 96 + 3],
                        xlos[i][:, c * n:(c + 1) * n],
                        start=False, stop=(c == DC - 1),
                        skip_group_check=True,
                    )
                nc.vector.tensor_scalar_add(lgT[:E, t0:t0 + n],
                                            dn27[:E, :n], rb[:])
                nc.vector.tensor_add(lgT[:E, t0:t0 + n],
                                     lgT[:E, t0:t0 + n],
                                     dn27[32:32 + E, :n])
                nc.scalar.activation(acts[:, t0:t0 + n], dn27[64:, :n],
                                     AF.Gelu)

            # batched softmax + top-2-of-3 renormalized combine weights:
            # comb_e = (p_e > p_min) * p_e / ((sum - min)/sum + 1e-6) / sum
            def softmax_block():
                lgtok = bp.tile([32, TC], F32, tag="lgtok")
                nc.vector.transpose(lgtok[:], lgT[:])
                ltv = lgtok[:].rearrange("p (b q) -> p b q", b=NBLK)[:, :, :E]
                probs = bp.tile([32, NBLK * E], F32, tag="probs")
                prv = probs[:].rearrange("p (b q) -> p b q", b=NBLK)
                nc.scalar.activation(prv, ltv, AF.Exp)
                ssum = bp.tile([32, NBLK], F32, tag="ssum")
                nc.vector.tensor_reduce(ssum[:], prv, axis=AX.X, op=ALU.add)
                pmin = bp.tile([32, NBLK], F32, tag="pmin")
                nc.vector.tensor_reduce(pmin[:], prv, axis=AX.X, op=ALU.min)
                rs = bp.tile([32, NBLK], F32, tag="rs")
                nc.vector.reciprocal(rs[:], ssum[:])
                den = bp.tile([32, NBLK], F32, tag="den")
                nc.vector.tensor_sub(den[:], ssum[:], pmin[:])
                nc.vector.tensor_mul(den[:], den[:], rs[:])
                nc.vector.tensor_scalar_add(den[:], den[:], 1e-6)
                invd = bp.tile([32, NBLK], F32, tag="invd")
                nc.vector.reciprocal(invd[:], den[:])
                t1 = bp.tile([32, NBLK], F32, tag="t1")
                nc.vector.tensor_mul(t1[:], rs[:], invd[:])
                combt = bp.tile([32, NBLK * 32], BF16, tag="combt")
                cbv = combt[:].rearrange("p (b q) -> p b q", b=NBLK)[:, :, :E]
                mask = bp.tile([32, NBLK * E], F32, tag="mask")
                mkv = mask[:].rearrange("p (b q) -> p b q", b=NBLK)
                pminb = pmin[:].unsqueeze(2).broadcast_to([32, NBLK, E])
                nc.vector.tensor_tensor(mkv, prv, pminb, op=ALU.is_gt)
                nc.vector.tensor_mul(mkv, mkv, prv)
                t1b = t1[:].unsqueeze(2).broadcast_to([32, NBLK, E])
                nc.vector.tensor_tensor(cbv, mkv, t1b, op=ALU.mult)
                combT = bp.tile([32, TC], BF16, tag="combT")
                nc.vector.transpose(combT[:], combt[:])
                return combT

            def expand_block(combT):
                for i2, n2 in enumerate(NT_SIZES):
                    tq = NT_OFF[i2]
                    ex = psH.tile([ER, 512], F32, tag="h", name=f"ex_{i2}")
                    nc.tensor.matmul(ex[:, :n2], bx[:],
                                     combT[:E, tq:tq + n2],
                                     start=True, stop=True)
                    nc.vector.tensor_mul(scaled[:, tq:tq + n2],
                                         acts[:, tq:tq + n2], ex[:, :n2])

            # ---- phase B: base MLP, out accumulates across all 24 j in
            # PSUM; phase A of tiles 1-3 and the softmax/expand interleave
            # into tile 0's j-loop so their inputs arrive under compute ----
            combT = None
            phase_a(0)
            for nt, n in enumerate(NT_SIZES):
                t0 = NT_OFF[nt]
                outp = [psO.tile([128, 512], F32, tag=f"out{m}",
                                 name=f"out{m}_{nt}")
                        for m in range(MC)]
                hsb_prev = None
                for j in range(HC + 1):
                    if nt == 0 and j in (4, 8, 12):
                        phase_a(j // 4)
                        if j == 12:
                            combT = softmax_block()
                    if nt == 0 and j == 19:
                        expand_block(combT)
                    if j < HC:
                        hps = psH.tile([128, 512], F32, tag="h",
                                       name=f"h_{nt}_{j}")
                        for c in range(DC):
                            o = (j * DC + c) * 128
                            nc.tensor.matmul(
                                hps[:, :n],
                                w1s[:, o:o + 128],
                                xts[nt][:, c * n:(c + 1) * n],
                                start=(c == 0), stop=(c == DC - 1),
                            )
                        hsb = hp.tile([128, 512], BF16, tag="hs",
                                      name=f"hs_{nt}_{j}")
                        nc.scalar.activation(
                            hsb[:, :n], hps[:, :n], AF.Gelu,
                            bias=b1[:, j:j + 1],
                        )
                    if j >= 1:
                        jj = j - 1
                        for m in range(MC):
                            nc.tensor.matmul(
                                outp[m][:, :n],
                                w2v[:, jj, m * 128:(m + 1) * 128],
                                hsb_prev[:, :n],
                                start=(jj == 0), stop=False,
                            )
                    hsb_prev = hsb
                # LoRA-up closes each PSUM accumulation group; the PSUM
                # evacuation + bias runs on DVE right behind each chunk
                last = nt == len(NT_SIZES) - 1
                osb = op.tile([128, MC * 512], F32, tag="osb",
                              name=f"osb_{nt}")
                for m in range(MC):
                    nc.tensor.matmul(
                        outp[m][:, :n],
                        wu[:, m * 128:(m + 1) * 128],
                        scaled[:, t0:t0 + n],
                        start=False, stop=True,
                    )
                    if last and m % 2 == 1:
                        nc.scalar.activation(
                            osb[:, m * 512:m * 512 + n], outp[m][:, :n],
                            AF.Identity, bias=b2[:, m:m + 1],
                        )
                    else:
                        nc.vector.tensor_scalar_add(
                            osb[:, m * 512:m * 512 + n], outp[m][:, :n],
                            b2[:, m:m + 1],
                        )
                oo = MC * t0
                odv = out_d[:, oo:oo + MC * n].rearrange(
                    "p (m t) -> p m t", m=MC)
                osv = osb[:].rearrange("p (m t) -> p m t", m=MC)
                if last:
                    nc.sync.dma_start(odv[:, :MC // 2, :],
                                      osv[:, :MC // 2, :n])
                    nc.sync.dma_start(odv[:, MC // 2:, :],
                                      osv[:, MC // 2:, :n])
                else:
                    nc.sync.dma_start(odv[:, :, :], osv[:, :, :n])

    nc.compile()
    return nc


def _pack_rwd(router_w, w_down):
    rw = np.asarray(router_w, np.float32)
    rw_hi = rw.astype(ml_dtypes.bfloat16).astype(np.float32)
    rwd = np.zeros((D, 96), ml_dtypes.bfloat16)
    rwd[:, :E] = rw_hi
    rwd[:, 32:32 + E] = rw - rw_hi
    rwd[:, 64:88] = np.asarray(w_down, np.float32).transpose(1, 0, 2).reshape(D, ER)
    # pack to SBUF layout [p, c, e]
    return np.ascontiguousarray(
        rwd.reshape(DC, 128, 96).transpose(1, 0, 2).reshape(128, DC * 96))


def _bf16(a):
    return np.ascontiguousarray(
        np.asarray(a, np.float32).astype(ml_dtypes.bfloat16))


def _pack_x(xT):
    # [D, TC] -> tile-major [128, sum(DC*n)]: per tile [p, c, t] contiguous
    blocks = []
    for i, n in enumerate(NT_SIZES):
        t0 = NT_OFF[i]
        blk = xT[:, t0:t0 + n].reshape(DC, 128, n).transpose(1, 0, 2)
        blocks.append(blk.reshape(128, DC * n))
    return np.ascontiguousarray(np.concatenate(blocks, axis=1))


def _prep_inputs(x, w1, b1, w2, b2, router_w, router_b, w_down, w_up):
    x = np.asarray(x, dtype=np.float32)
    xT = x.reshape(T, D).T  # [D, T]
    w1p = _bf16(w1).reshape(DC, 128, HC, 128).transpose(1, 2, 0, 3)
    w2p = _bf16(w2).reshape(HC, 128, D).transpose(1, 0, 2)
    common = {
        "w1": np.ascontiguousarray(w1p.reshape(128, HC * DC * 128)),
        "w2": np.ascontiguousarray(w2p.reshape(128, HC * D)),
        "wu": _bf16(np.asarray(w_up, np.float32).reshape(ER, D)),
        "b1r": np.ascontiguousarray(
            np.asarray(b1, np.float32).reshape(HC, 128).T),
        "b2r": np.ascontiguousarray(
            np.asarray(b2, np.float32).reshape(MC, 128).T),
        "rwd": _pack_rwd(router_w, w_down),
        "rb": np.ascontiguousarray(
            np.asarray(router_b, np.float32).reshape(E, 1)),
        "bexp": _bf16(np.repeat(np.eye(E, dtype=np.float32), R, axis=1)),
    }
    xT_hi = xT.astype(ml_dtypes.bfloat16)
    xT_lo = (xT - xT_hi.astype(np.float32)).astype(ml_dtypes.bfloat16)
    in_maps = []
    for c in range(NCORES):
        m = dict(common)
        m["xt"] = _pack_x(xT_hi[:, c * TC:(c + 1) * TC])
        m["xlo"] = _pack_x(xT_lo[:, c * TC:(c + 1) * TC])
        in_maps.append(m)
    return in_maps


def _run(inputs, trace=False):
    if "nc" not in _cache:
        _cache["nc"] = _build()
    nc = _cache["nc"]
    in_maps = _prep_inputs(**inputs)
    res = run_bass_kernel_spmd(nc, in_maps, core_ids=list(range(NCORES)),
                               trace=trace)
    # unpack tile-major [128, MC*TC] per core -> [D, T] -> [B, N, D]
    cols = []
    for c in range(NCORES):
        arr = res.results[c]["outT"]
        for i, n in enumerate(NT_SIZES):
            oo = MC * NT_OFF[i]
            blk = arr[:, oo:oo + MC * n].reshape(128, MC, n)
            cols.append(blk.transpose(1, 0, 2).reshape(D, n))
    outT = np.concatenate(cols, axis=1)  # [D, T]
    out = np.ascontiguousarray(outT.T).reshape(B, N, D).astype(np.float32)
    return out, res


def kernel(**inputs):
    return _run(inputs)[0]


# revision 25
# speedup vs baseline: 1.0134x; 1.0100x over previous
"""TRN2 Bass kernel for ConvNeXt-MLP + parallel top-2-of-3 LoRA-MoE.

Data-parallel over the token dim across 8 NeuronCores (12544 tokens ->
1568/core). All weights replicated. Per core, everything is computed in
feature-major ("transposed") layout: activations live in SBUF as
[features_on_partitions, tokens_on_free_dim]; the host transposes x in and
the output back out.

All matmuls run in bf16 (1 cycle/row on the PE; the 2-byte LDWEIGHTS hides
under the matmul, unlike the 4-byte f32r weight load). w1 and w2 are fully
SBUF-resident in bf16 (9.4 MB), so the hidden dim is a single 24-chunk pass
per token tile with the output accumulating entirely in PSUM.

Router exactness: bf16 alone flips the top-2 selection on ~8 near-tie
tokens (each flip is a ~0.1 abs output error). The kernel therefore
computes logits as x_hi@rw_hi + x_hi@rw_lo + x_lo@rw_hi with bf16 hi/lo
splits of both operands (max logit err ~2e-5 vs the f32 reference, smallest
top-2 margin in-distribution is 5.3e-5 -> selection is bit-identical).
rw_lo/lora-down live at PSUM quadrant offsets 32/64 (engine partition-base
rule). Softmax + top-2 + renormalize run fully batched on DVE via one
32x32 stream transpose each way.

Scheduling: one need-ordered DMA stream on the sync queue (xt0, rwd, xlo0,
then w1/w2 j-chunks interleaved with the remaining xt/xlo tiles in exact
consumption order - the queues fair-share HBM bandwidth, so priority =
order). Phase A for tiles 1-3 is interleaved INTO tile 0's j-loop (at
j=4/8/12) so their x DMAs arrive under compute. PSUM->SBUF output copies
run on the otherwise-idle Vector engine, interleaved with the LoRA-up
matmuls per output chunk.
"""

import numpy as np
import ml_dtypes

import concourse.bacc as bacc
import concourse.mybir as mybir
import concourse.tile as tile
from concourse.bass_utils import run_bass_kernel_spmd

F32 = mybir.dt.float32
BF16 = mybir.dt.bfloat16
AF = mybir.ActivationFunctionType
ALU = mybir.AluOpType
AX = mybir.AxisListType

NCORES = 8
B, N, D = 64, 196, 768
T = B * N                  # 12544 tokens total
TC = T // NCORES           # 1568 tokens per core
HID = 4 * D                # 3072
E, R = 3, 8
ER = E * R                 # 24
DC = D // 128              # 6 input-feature chunks
HC = HID // 128            # 24 hidden chunks
MC = D // 128              # 6 output chunks
NT_SIZES = [448, 448, 448, 224]    # token tiles per core (sum = 1568)
NT_OFF = [0, 448, 896, 1344]
NBLK = TC // 32            # 49 32-token blocks for the stream transpose
# w1/w2 are DMA'd in j-chunks in consumption order, interleaved with the
# xt/xlo tiles of later token tiles
WCHUNKS = [(0, 2), (2, 4), (4, 8), (8, 12), (12, 17), (17, 24)]

_cache = {}


def _build():
    nc = bacc.Bacc("TRN2", target_bir_lowering=False, debug=False)

    # all inputs are host-packed to the exact SBUF layout so every DMA is
    # a straight [128, X] copy with multi-KB rows at full HBM bandwidth
    xt_d = nc.dram_tensor("xt", [128, DC * TC], BF16, kind="ExternalInput")
    xlo_d = nc.dram_tensor("xlo", [128, DC * TC], BF16, kind="ExternalInput")
    w1_d = nc.dram_tensor("w1", [128, HC * DC * 128], BF16,
                          kind="ExternalInput")
    w2_d = nc.dram_tensor("w2", [128, HC * D], BF16, kind="ExternalInput")
    wu_d = nc.dram_tensor("wu", [ER, D], BF16, kind="ExternalInput")
    b1_d = nc.dram_tensor("b1r", [128, HC], F32, kind="ExternalInput")
    b2_d = nc.dram_tensor("b2r", [128, MC], F32, kind="ExternalInput")
    rwd_d = nc.dram_tensor("rwd", [128, DC * 96], BF16, kind="ExternalInput")
    rb_d = nc.dram_tensor("rb", [E, 1], F32, kind="ExternalInput")
    bx_d = nc.dram_tensor("bexp", [E, ER], BF16, kind="ExternalInput")
    out_d = nc.dram_tensor("outT", [128, MC * TC], F32,
                           kind="ExternalOutput")

    with tile.TileContext(nc) as tc:
        with (
            tc.tile_pool(name="const", bufs=1) as cp,
            tc.tile_pool(name="big", bufs=1) as bp,
            tc.tile_pool(name="hbuf", bufs=3) as hp,
            tc.tile_pool(name="osb", bufs=2) as op,
            tc.tile_pool(name="psO", bufs=1, space="PSUM") as psO,
            tc.tile_pool(name="psH", bufs=2, space="PSUM") as psH,
        ):
            # ---- tiles ----
            xts = [bp.tile([128, DC * n], BF16, tag=f"xt{i}", name=f"xt{i}")
                   for i, n in enumerate(NT_SIZES)]
            xlos = [bp.tile([128, DC * n], BF16, tag=f"xlo{i}",
                            name=f"xlo{i}")
                    for i, n in enumerate(NT_SIZES)]
            rwd = cp.tile([128, DC * 96], BF16, tag="rwd")
            wu = cp.tile([ER, D], BF16, tag="wu")
            b1 = cp.tile([128, HC], F32, tag="b1")
            b2 = cp.tile([128, MC], F32, tag="b2")
            rb = cp.tile([E, 1], F32, tag="rb")
            bx = cp.tile([E, ER], BF16, tag="bx")
            w1s = bp.tile([128, HC * DC * 128], BF16, tag="w1s")
            w2s = bp.tile([128, HC * D], BF16, tag="w2s")
            w2v = w2s[:].rearrange("p (j f) -> p j f", j=HC)
            lgT = bp.tile([32, TC], F32, tag="lgT")
            acts = bp.tile([ER, TC], F32, tag="acts")
            scaled = bp.tile([ER, TC], BF16, tag="scaled")

            def load_x(i):
                lo = DC * NT_OFF[i]
                hi = lo + DC * NT_SIZES[i]
                nc.sync.dma_start(xts[i][:], xt_d[:, lo:hi])
                nc.sync.dma_start(xlos[i][:], xlo_d[:, lo:hi])

            def load_w(q):
                j0, j1 = WCHUNKS[q]
                nc.sync.dma_start(w1s[:, j0 * DC * 128:j1 * DC * 128],
                                  w1_d[:, j0 * DC * 128:j1 * DC * 128])
                nc.sync.dma_start(w2s[:, j0 * D:j1 * D],
                                  w2_d[:, j0 * D:j1 * D])

            # need-ordered single DMA stream (sync); tiny consts on gpsimd.
            # xt0/xlo0 stream in c-chunk pieces so phase A's contraction loop
            # starts on the first chunk right as the PE warmup ends
            n0 = NT_SIZES[0]
            nc.sync.dma_start(rwd[:], rwd_d[:])
            for c0, c1 in ((0, 2), (2, 4), (4, 6)):
                nc.sync.dma_start(xts[0][:, c0 * n0:c1 * n0],
                                  xt_d[:, c0 * n0:c1 * n0])
            for c0, c1 in ((0, 3), (3, 6)):
                nc.sync.dma_start(xlos[0][:, c0 * n0:c1 * n0],
                                  xlo_d[:, c0 * n0:c1 * n0])
            nc.sync.dma_start(w1s[:, 0:2 * DC * 128],
                              w1_d[:, 0:2 * DC * 128])
            nc.sync.dma_start(w2s[:, 0:2 * D], w2_d[:, 0:2 * D])
            nc.gpsimd.dma_start(rb[:], rb_d[:])
            nc.gpsimd.dma_start(b1[:], b1_d[:])
            nc.gpsimd.dma_start(b2[:], b2_d[:])
            nc.gpsimd.dma_start(bx[:], bx_d[:])
            nc.gpsimd.dma_start(wu[:], wu_d[:])
            load_w(1)
            load_x(1)
            load_w(2)
            load_x(2)
            load_w(3)
            load_x(3)
            load_w(4)
            load_w(5)

            # warm up the PE DVFS ramp with dependency-free tiny matmuls so
            # the first real matmuls run at full clock
            warm = cp.tile([16, 16], BF16, tag="warm")
            nc.vector.memset(warm[:], 0.0)
            wps = psH.tile([16, 512], F32, tag="h", name="warm_ps")
            for k in range(28):
                nc.tensor.matmul(wps[:16, (k % 10) * 16:(k % 10) * 16 + 16],
                                 warm[:], warm[:], start=True, stop=True,
                                 skip_group_check=True)

            nc.vector.memset(lgT[:], 0.0)

            # ---- phase A (per tile): router logits + LoRA activations ----
            def phase_a(i):
                n, t0 = NT_SIZES[i], NT_OFF[i]
                dn27 = psH.tile([88, 512], F32, tag="h", name=f"dn27_{i}")
                for c in range(DC):
                    nc.tensor.matmul(
                        dn27[:, :n],
                        rwd[:, c * 96:c * 96 + 88],
                        xts[i][:, c * n:(c + 1) * n],
                        start=(c == 0), stop=(c == DC - 1),
                    )
                for c in range(DC):
                    nc.tensor.matmul(
                        dn27[:3, :n],
                        rwd[:, c * 96:c * 96 + 3],
                        xlos[i][:, c * n:(c + 1) * n],
                        start=False, stop=(c == DC - 1),
                        skip_group_check=True,
                    )
                nc.vector.tensor_scalar_add(lgT[:E, t0:t0 + n],
                                            dn27[:E, :n], rb[:])
                nc.vector.tensor_add(lgT[:E, t0:t0 + n],
                                     lgT[:E, t0:t0 + n],
                                     dn27[32:32 + E, :n])
                nc.scalar.activation(acts[:, t0:t0 + n], dn27[64:, :n],
                                     AF.Gelu)

            # batched softmax + top-2-of-3 renormalized combine weights:
            # comb_e = (p_e > p_min) * p_e / ((sum - min)/sum + 1e-6) / sum
            def softmax_block():
                lgtok = bp.tile([32, TC], F32, tag="lgtok")
                nc.vector.transpose(lgtok[:], lgT[:])
                ltv = lgtok[:].rearrange("p (b q) -> p b q", b=NBLK)[:, :, :E]
                probs = bp.tile([32, NBLK * E], F32, tag="probs")
                prv = probs[:].rearrange("p (b q) -> p b q", b=NBLK)
                nc.scalar.activation(prv, ltv, AF.Exp)
                ssum = bp.tile([32, NBLK], F32, tag="ssum")
                nc.vector.tensor_reduce(ssum[:], prv, axis=AX.X, op=ALU.add)
                pmin = bp.tile([32, NBLK], F32, tag="pmin")
                nc.vector.tensor_reduce(pmin[:], prv, axis=AX.X, op=ALU.min)
                rs = bp.tile([32, NBLK], F32, tag="rs")
                nc.vector.reciprocal(rs[:], ssum[:])
                den = bp.tile([32, NBLK], F32, tag="den")
                nc.vector.tensor_sub(den[:], ssum[:], pmin[:])
                nc.vector.tensor_mul(den[:], den[:], rs[:])
                nc.vector.tensor_scalar_add(den[:], den[:], 1e-6)
                invd = bp.tile([32, NBLK], F32, tag="invd")
                nc.vector.reciprocal(invd[:], den[:])
                t1 = bp.tile([32, NBLK], F32, tag="t1")
                nc.vector.tensor_mul(t1[:], rs[:], invd[:])
                combt = bp.tile([32, NBLK * 32], BF16, tag="combt")
                cbv = combt[:].rearrange("p (b q) -> p b q", b=NBLK)[:, :, :E]
                mask = bp.tile([32, NBLK * E], F32, tag="mask")
                mkv = mask[:].rearrange("p (b q) -> p b q", b=NBLK)
                pminb = pmin[:].unsqueeze(2).broadcast_to([32, NBLK, E])
                nc.vector.tensor_tensor(mkv, prv, pminb, op=ALU.is_gt)
                nc.vector.tensor_mul(mkv, mkv, prv)
                t1b = t1[:].unsqueeze(2).broadcast_to([32, NBLK, E])
                nc.vector.tensor_tensor(cbv, mkv, t1b, op=ALU.mult)
                combT = bp.tile([32, TC], BF16, tag="combT")
                nc.vector.transpose(combT[:], combt[:])
                return combT

            def expand_block(combT):
                for i2, n2 in enumerate(NT_SIZES):
                    tq = NT_OFF[i2]
                    ex = psH.tile([ER, 512], F32, tag="h", name=f"ex_{i2}")
                    nc.tensor.matmul(ex[:, :n2], bx[:],
                                     combT[:E, tq:tq + n2],
                                     start=True, stop=True)
                    nc.vector.tensor_mul(scaled[:, tq:tq + n2],
                                         acts[:, tq:tq + n2], ex[:, :n2])

            # ---- phase B: base MLP, out accumulates across all 24 j in
            # PSUM; phase A of tiles 1-3 and the softmax/expand interleave
            # into tile 0's j-loop so their inputs arrive under compute ----
            combT = None
            phase_a(0)
            for nt, n in enumerate(NT_SIZES):
                t0 = NT_OFF[nt]
                outp = [psO.tile([128, 512], F32, tag=f"out{m}",
                                 name=f"out{m}_{nt}")
                        for m in range(MC)]
                hsb_prev = None
                for j in range(HC + 1):
                    if nt == 0 and j in (4, 8, 12):
                        phase_a(j // 4)
                        if j == 12:
                            combT = softmax_block()
                    if nt == 0 and j == 19:
                        expand_block(combT)
                    if j < HC:
                        hps = psH.tile([128, 512], F32, tag="h",
                                       name=f"h_{nt}_{j}")
                        for c in range(DC):
                            o = (j * DC + c) * 128
                            nc.tensor.matmul(
                                hps[:, :n],
                                w1s[:, o:o + 128],
                                xts[nt][:, c * n:(c + 1) * n],
                                start=(c == 0), stop=(c == DC - 1),
                            )
                        hsb = hp.tile([128, 512], BF16, tag="hs",
                                      name=f"hs_{nt}_{j}")
                        nc.scalar.activation(
                            hsb[:, :n], hps[:, :n], AF.Gelu,
                            bias=b1[:, j:j + 1],
                        )
                    if j >= 1:
                        jj = j - 1
                        for m in range(MC):
                            nc.tensor.matmul(
                                outp[m][:, :n],
                                w2v[:, jj, m * 128:(m + 1) * 128],
                                hsb_prev[:, :n],
                                start=(jj == 0), stop=False,
                            )
                    hsb_prev = hsb
                # LoRA-up closes each PSUM accumulation group; the PSUM
                # evacuation + bias runs on DVE right behind each chunk
                last = nt == len(NT_SIZES) - 1
                osb = op.tile([128, MC * 512], F32, tag="osb",
                              name=f"osb_{nt}")
                for m in range(MC):
                    nc.tensor.matmul(
                        outp[m][:, :n],
                        wu[:, m * 128:(m + 1) * 128],
                        scaled[:, t0:t0 + n],
                        start=False, stop=True,
                    )
                    if last and m % 2 == 1:
                        nc.scalar.activation(
                            osb[:, m * 512:m * 512 + n], outp[m][:, :n],
                            AF.Identity, bias=b2[:, m:m + 1],
                        )
                    else:
                        nc.vector.tensor_scalar_add(
                            osb[:, m * 512:m * 512 + n], outp[m][:, :n],
                            b2[:, m:m + 1],
                        )
                oo = MC * t0
                odv = out_d[:, oo:oo + MC * n].rearrange(
                    "p (m t) -> p m t", m=MC)
                osv = osb[:].rearrange("p (m t) -> p m t", m=MC)
                if last:
                    nc.sync.dma_start(odv[:, :MC // 2, :],
                                      osv[:, :MC // 2, :n])
                    nc.sync.dma_start(odv[:, MC // 2:, :],
                                      osv[:, MC // 2:, :n])
                else:
                    nc.sync.dma_start(odv[:, :, :], osv[:, :, :n])

    nc.compile()
    return nc


def _pack_rwd(router_w, w_down):
    rw = np.asarray(router_w, np.float32)
    rw_hi = rw.astype(ml_dtypes.bfloat16).astype(np.float32)
    rwd = np.zeros((D, 96), ml_dtypes.bfloat16)
    rwd[:, :E] = rw_hi
    rwd[:, 32:32 + E] = rw - rw_hi
    rwd[:, 64:88] = np.asarray(w_down, np.float32).transpose(1, 0, 2).reshape(D, ER)
    # pack to SBUF layout [p, c, e]
    return np.ascontiguousarray(
        rwd.reshape(DC, 128, 96).transpose(1, 0, 2).reshape(128, DC * 96))


def _bf16(a):
    return np.ascontiguousarray(
        np.asarray(a, np.float32).astype(ml_dtypes.bfloat16))


def _pack_x(xT):
    # [D, TC] -> tile-major [128, sum(DC*n)]: per tile [p, c, t] contiguous
    blocks = []
    for i, n in enumerate(NT_SIZES):
        t0 = NT_OFF[i]
        blk = xT[:, t0:t0 + n].reshape(DC, 128, n).transpose(1, 0, 2)
        blocks.append(blk.reshape(128, DC * n))
    return np.ascontiguousarray(np.concatenate(blocks, axis=1))


def _prep_inputs(x, w1, b1, w2, b2, router_w, router_b, w_down, w_up):
    x = np.asarray(x, dtype=np.float32)
    xT = x.reshape(T, D).T  # [D, T]
    w1p = _bf16(w1).reshape(DC, 128, HC, 128).transpose(1, 2, 0, 3)
    w2p = _bf16(w2).reshape(HC, 128, D).transpose(1, 0, 2)
    common = {
        "w1": np.ascontiguousarray(w1p.reshape(128, HC * DC * 128)),
        "w2": np.ascontiguousarray(w2p.reshape(128, HC * D)),
        "wu": _bf16(np.asarray(w_up, np.float32).reshape(ER, D)),
        "b1r": np.ascontiguousarray(
            np.asarray(b1, np.float32).reshape(HC, 128).T),
        "b2r": np.ascontiguousarray(
            np.asarray(b2, np.float32).reshape(MC, 128).T),
        "rwd": _pack_rwd(router_w, w_down),
        "rb": np.ascontiguousarray(
            np.asarray(router_b, np.float32).reshape(E, 1)),
        "bexp": _bf16(np.repeat(np.eye(E, dtype=np.float32), R, axis=1)),
    }
    xT_hi = xT.astype(ml_dtypes.bfloat16)
    xT_lo = (xT - xT_hi.astype(np.float32)).astype(ml_dtypes.bfloat16)
    in_maps = []
    for c in range(NCORES):
        m = dict(common)
        m["xt"] = _pack_x(xT_hi[:, c * TC:(c + 1) * TC])
        m["xlo"] = _pack_x(xT_lo[:, c * TC:(c + 1) * TC])
        in_maps.append(m)
    return in_maps


def _run(inputs, trace=False):
    if "nc" not in _cache:
        _cache["nc"] = _build()
    nc = _cache["nc"]
    in_maps = _prep_inputs(**inputs)
    res = run_bass_kernel_spmd(nc, in_maps, core_ids=list(range(NCORES)),
                               trace=trace)
    # unpack tile-major [128, MC*TC] per core -> [D, T] -> [B, N, D]
    cols = []
    for c in range(NCORES):
        arr = res.results[c]["outT"]
        for i, n in enumerate(NT_SIZES):
            oo = MC * NT_OFF[i]
            blk = arr[:, oo:oo + MC * n].reshape(128, MC, n)
            cols.append(blk.transpose(1, 0, 2).reshape(D, n))
    outT = np.concatenate(cols, axis=1)  # [D, T]
    out = np.ascontiguousarray(outT.T).reshape(B, N, D).astype(np.float32)
    return out, res


def kernel(**inputs):
    return _run(inputs)[0]
